# revision 4
# baseline (speedup 1.0000x reference)
"""Trainium2 Bass kernel for nn_EstimatorNetwork (gnn_message_passing).

Mathematical reformulation: each candidate anchor (f_b, n_b) perturbs a shared
linear recurrence by a rank-1 kill, so

    total(b) = S_base - X[f_b, n_b] * U[f_b, n_b]

where X is the forward chain  x_f = K_f * (b_f + W_{f-1} @ x_{f-1})
and   U the backward chain    u_f = 1 + W_f^T (K_{f+1} * u_{f+1}),
K the selected-anchor keep mask, S_base = sum(X).

Device layout: 8 cores each own a 288-row slice of both chains. Per frame:
a weight-streaming matvec slice on the PE (vector stationary, 18 matmuls of
N=288), per-core AllGather rebuilds the full 2304-vector in a DRAM table,
PE-transpose prepares the next stationary. Final: indirect-DMA gather of the
1024 candidate (X, U) pairs + rank-1 combine.
"""
import sys

if "/opt/trn_rl_repo" not in sys.path:
    sys.path.insert(0, "/opt/trn_rl_repo")

import numpy as np

import concourse.bass as bass
import concourse.bacc as bacc
import concourse.mybir as mybir
import concourse.tile as tile
from concourse.bass_utils import run_bass_kernel_spmd
from concourse.masks import make_identity

NCORES = 8
NBR = 64            # blocks per row (node = row*64 + col)
N = 2304            # nodes per frame
F = 32              # frames
B = 1024            # candidates
RS = N // NCORES    # 288 rows per core per chain
JT = N // 128       # 18 k-tiles
BC = B // NCORES    # 128 candidates per core

FP32 = mybir.dt.float32
INT32 = mybir.dt.int32

_PROGRAM = None


def _build_program():
    nc = bacc.Bacc("TRN2", target_bir_lowering=False, debug=False,
                   num_devices=NCORES)

    # ---- per-core external inputs ----
    wf_d = nc.dram_tensor("wf", [F - 1, N, RS], FP32, kind="ExternalInput")
    wb_d = nc.dram_tensor("wb", [F - 1, N, RS], FP32, kind="ExternalInput")
    bsl_d = nc.dram_tensor("bsl", [F, RS], FP32, kind="ExternalInput")
    ksl_d = nc.dram_tensor("ksl", [F, RS], FP32, kind="ExternalInput")
    kpre_d = nc.dram_tensor("kpre", [F - 1, N], FP32, kind="ExternalInput")
    xidx_d = nc.dram_tensor("xidx", [BC, 1], INT32, kind="ExternalInput")
    uidx_d = nc.dram_tensor("uidx", [BC, 1], INT32, kind="ExternalInput")
    out_d = nc.dram_tensor("out", [BC, 1], FP32, kind="ExternalOutput")

    # ---- internal DRAM: AllGather landing tables (x and v = u reversed) ----
    tabx = nc.dram_tensor("tabx", [F * N], FP32)   # tabx[t*N : ] = x_t
    tabv = nc.dram_tensor("tabv", [F * N], FP32)   # tabv[t*N : ] = u_{31-t}

    groups = [list(range(NCORES))]

    with tile.TileContext(nc) as tc:
        with (
            tc.tile_pool(name="const", bufs=1) as cpool,
            tc.tile_pool(name="wpool", bufs=2) as wpool,
            tc.tile_pool(name="sb", bufs=2) as sb,
            tc.tile_pool(name="ps", bufs=2, space="PSUM") as ps,
            tc.tile_pool(name="ps1", bufs=1, space="PSUM") as ps1,
            tc.tile_pool(name="agdram", bufs=2, space="DRAM") as agdram,
        ):
            # constants
            ident = cpool.tile([JT, JT], FP32)
            make_identity(nc, ident[:])
            acc = cpool.tile([JT, 128], FP32, tag="acc")
            nc.gpsimd.memset(acc[:], 0.0)
            kpre_sb = cpool.tile([JT, (F - 1) * 128], FP32, tag="kpre")
            nc.sync.dma_start(
                kpre_sb[:].rearrange("j (t l) -> j t l", t=F - 1),
                kpre_d[:].rearrange("t (j l) -> j t l", j=JT),
            )

            # stationaries for the next step (produced by round t, consumed t+1)
            xT = None
            vT = None

            for t in range(F):
                # ---------- compute this round's slices ----------
                if t == 0:
                    bslab = sb.tile([1, RS], FP32, tag="bslab")
                    kslab = sb.tile([1, RS], FP32, tag="kslab")
                    nc.sync.dma_start(bslab[:], bsl_d[0].unsqueeze(0))
                    nc.sync.dma_start(kslab[:], ksl_d[0].unsqueeze(0))
                    xsl = sb.tile([1, RS], FP32, tag="xsl")
                    nc.vector.tensor_mul(xsl[:], bslab[:], kslab[:])
                    vsl = sb.tile([1, RS], FP32, tag="vsl")
                    nc.gpsimd.memset(vsl[:], 1.0)
                else:
                    # fwd matvec slice: psx[0, n] = sum_j x_{t-1}[j] * W[t-1][Rc+n, j]
                    wf_t = wpool.tile([128, JT, RS], FP32, tag="wf")
                    nc.sync.dma_start(
                        wf_t[:], wf_d[t - 1].rearrange("(j p) n -> p j n", p=128)
                    )
                    psx = ps.tile([1, RS], FP32, tag="psx")
                    for j in range(JT):
                        nc.tensor.matmul(
                            psx[:], xT[:, j:j + 1], wf_t[:, j, :],
                            start=(j == 0), stop=(j == JT - 1),
                        )
                    bslab = sb.tile([1, RS], FP32, tag="bslab")
                    kslab = sb.tile([1, RS], FP32, tag="kslab")
                    nc.sync.dma_start(bslab[:], bsl_d[t].unsqueeze(0))
                    nc.sync.dma_start(kslab[:], ksl_d[t].unsqueeze(0))
                    xsl = sb.tile([1, RS], FP32, tag="xsl")
                    nc.vector.tensor_add(xsl[:], psx[:], bslab[:])
                    nc.vector.tensor_mul(xsl[:], xsl[:], kslab[:])

                # x AllGather
                aginx = agdram.tile([1, RS], FP32, tag="aginx")
                nc.sync.dma_start(aginx[:], xsl[:])
                nc.gpsimd.collective_compute(
                    "AllGather", mybir.AluOpType.bypass, replica_groups=groups,
                    ins=[aginx[:]], outs=[tabx[t * N:(t + 1) * N]],
                )

                if t > 0:
                    # bwd matvec slice: psv[0, n] = sum_i (K v)[i] * W[31-t][i, Rc+n]
                    wb_t = wpool.tile([128, JT, RS], FP32, tag="wb")
                    nc.sync.dma_start(
                        wb_t[:], wb_d[t - 1].rearrange("(j p) n -> p j n", p=128)
                    )
                    psv = ps.tile([1, RS], FP32, tag="psv")
                    for j in range(JT):
                        nc.tensor.matmul(
                            psv[:], vT[:, j:j + 1], wb_t[:, j, :],
                            start=(j == 0), stop=(j == JT - 1),
                        )
                    vsl = sb.tile([1, RS], FP32, tag="vsl")
                    nc.vector.tensor_scalar_add(vsl[:], psv[:], 1.0)

                # v AllGather
                aginv = agdram.tile([1, RS], FP32, tag="aginv")
                nc.sync.dma_start(aginv[:], vsl[:])
                nc.gpsimd.collective_compute(
                    "AllGather", mybir.AluOpType.bypass, replica_groups=groups,
                    ins=[aginv[:]], outs=[tabv[t * N:(t + 1) * N]],
                )

                # ---------- prep next stationaries from gathered vectors ----------
                x_nat = sb.tile([JT, 128], FP32, tag="x_nat")
                nc.sync.dma_start(
                    x_nat[:], tabx[t * N:(t + 1) * N].rearrange("(j l) -> j l", j=JT)
                )
                nc.vector.tensor_add(acc[:], acc[:], x_nat[:])
                if t < F - 1:
                    pstx = ps1.tile([128, JT], FP32, tag="pstx")
                    nc.tensor.transpose(pstx[:], x_nat[:], ident[:])
                    xT = sb.tile([128, JT], FP32, tag="xT")
                    nc.vector.tensor_copy(xT[:], pstx[:])

                    v_nat = sb.tile([JT, 128], FP32, tag="v_nat")
                    nc.sync.dma_start(
                        v_nat[:],
                        tabv[t * N:(t + 1) * N].rearrange("(j l) -> j l", j=JT),
                    )
                    # premask with K[31-t] before transposing (stationary for t+1)
                    nc.vector.tensor_mul(
                        v_nat[:], v_nat[:], kpre_sb[:, t * 128:(t + 1) * 128]
                    )
                    pstv = ps1.tile([128, JT], FP32, tag="pstv")
                    nc.tensor.transpose(pstv[:], v_nat[:], ident[:])
                    vT = sb.tile([128, JT], FP32, tag="vT")
                    nc.vector.tensor_copy(vT[:], pstv[:])

            # ---------- finale: S_base broadcast + candidate gather ----------
            red = sb.tile([JT, 1], FP32, tag="red")
            nc.vector.tensor_reduce(red[:], acc[:], mybir.AxisListType.X,
                                    mybir.AluOpType.add)
            ones = cpool.tile([JT, 128], FP32, tag="ones")
            nc.gpsimd.memset(ones[:], 1.0)
            ps_sb = ps1.tile([128, 1], FP32, tag="ps_sb")
            nc.tensor.matmul(ps_sb[:], ones[:], red[:], start=True, stop=True)

            idx_x = sb.tile([BC, 1], INT32, tag="idx_x")
            idx_u = sb.tile([BC, 1], INT32, tag="idx_u")
            nc.sync.dma_start(idx_x[:], xidx_d[:])
            nc.sync.dma_start(idx_u[:], uidx_d[:])
            gx = sb.tile([BC, 1], FP32, tag="gx")
            gu = sb.tile([BC, 1], FP32, tag="gu")
            nc.gpsimd.indirect_dma_start(
                out=gx[:], out_offset=None,
                in_=tabx[:].rearrange("(a b) -> a b", b=1),
                in_offset=bass.IndirectOffsetOnAxis(ap=idx_x[:, :1], axis=0),
            )
            nc.gpsimd.indirect_dma_start(
                out=gu[:], out_offset=None,
                in_=tabv[:].rearrange("(a b) -> a b", b=1),
                in_offset=bass.IndirectOffsetOnAxis(ap=idx_u[:, :1], axis=0),
            )
            prod = sb.tile([BC, 1], FP32, tag="prod")
            nc.vector.tensor_mul(prod[:], gx[:], gu[:])
            outv = sb.tile([BC, 1], FP32, tag="outv")
            nc.vector.tensor_sub(outv[:], ps_sb[:], prod[:])
            nc.sync.dma_start(out_d[:], outv[:])

    nc.compile()
    return nc


def _get_program():
    global _PROGRAM
    if _PROGRAM is None:
        _PROGRAM = _build_program()
    return _PROGRAM


def _host_prep(weights, biases, selected_anchor_points, candidate_anchor_points):
    W = np.ascontiguousarray(weights, dtype=np.float32)
    Bi = np.ascontiguousarray(biases, dtype=np.float32)
    sel = np.asarray(selected_anchor_points)
    cand = np.asarray(candidate_anchor_points)

    K = np.ones((F, N), dtype=np.float32)
    K[sel[:, 0], sel[:, 1] * NBR + sel[:, 2]] = 0.0

    cf = cand[:, 0].astype(np.int64)
    cn = (cand[:, 1] * NBR + cand[:, 2]).astype(np.int64)
    xidx = (cf * N + cn).astype(np.int32)
    uidx = ((F - 1 - cf) * N + cn).astype(np.int32)

    Wrev = W[::-1]  # Wrev[s] = W[30-s]
    kpre = np.ascontiguousarray(K[F - 1:0:-1])  # kpre[s] = K[31-s], s=0..30

    in_maps = []
    for c in range(NCORES):
        rows = slice(RS * c, RS * (c + 1))
        wf_c = np.ascontiguousarray(W[:, rows, :].transpose(0, 2, 1))
        wb_c = np.ascontiguousarray(Wrev[:, :, rows])
        in_maps.append({
            "wf": wf_c,
            "wb": wb_c,
            "bsl": np.ascontiguousarray(Bi[:, rows]),
            "ksl": np.ascontiguousarray(K[:, rows]),
            "kpre": kpre,
            "xidx": xidx[BC * c: BC * (c + 1)].reshape(BC, 1),
            "uidx": uidx[BC * c: BC * (c + 1)].reshape(BC, 1),
        })
    return in_maps


def kernel(weights, biases, selected_anchor_points, candidate_anchor_points):
    nc = _get_program()
    in_maps = _host_prep(weights, biases, selected_anchor_points,
                         candidate_anchor_points)
    last_err = None
    for _attempt in range(2):
        try:
            res = run_bass_kernel_spmd(nc, in_maps,
                                       core_ids=list(range(NCORES)))
            break
        except Exception as e:  # transient device flake: retry once
            last_err = e
    else:
        raise last_err
    out = np.concatenate(
        [res.results[c]["out"].reshape(BC) for c in range(NCORES)]
    ).astype(np.float32)
    return out


# revision 5
# speedup vs baseline: 1.6737x; 1.6737x over previous
"""Trainium2 Bass kernel for nn_EstimatorNetwork (gnn_message_passing).

Mathematical reformulation: each candidate anchor (f_b, n_b) perturbs a shared
linear recurrence by a rank-1 kill, so

    total(b) = S_base - X[f_b, n_b] * U[f_b, n_b]

where X is the forward chain  x_f = K_f * (b_f + W_{f-1} @ x_{f-1})
and   U the backward chain    u_f = 1 + W_f^T (K_{f+1} * u_{f+1}),
K the selected-anchor keep mask, S_base = sum(X).

Device layout: 8 cores each own a 288-row slice of both chains. Per frame:
a weight-streaming matvec slice on the PE (vector stationary, 18 matmuls of
N=288), per-core AllGather rebuilds the full 2304-vector in a DRAM table,
PE-transpose prepares the next stationary. Final: indirect-DMA gather of the
1024 candidate (X, U) pairs + rank-1 combine.
"""
import sys

if "/opt/trn_rl_repo" not in sys.path:
    sys.path.insert(0, "/opt/trn_rl_repo")

import numpy as np

import concourse.bass as bass
import concourse.bacc as bacc
import concourse.mybir as mybir
import concourse.tile as tile
from concourse.bass_utils import run_bass_kernel_spmd
from concourse.masks import make_identity

NCORES = 8
NBR = 64            # blocks per row (node = row*64 + col)
N = 2304            # nodes per frame
F = 32              # frames
B = 1024            # candidates
RS = N // NCORES    # 288 rows per core per chain
JT = N // 128       # 18 k-tiles
BC = B // NCORES    # 128 candidates per core

FP32 = mybir.dt.float32
BF16 = mybir.dt.bfloat16
INT32 = mybir.dt.int32

_PROGRAM = None


def _build_program():
    nc = bacc.Bacc("TRN2", target_bir_lowering=False, debug=False,
                   num_devices=NCORES)

    # ---- per-core external inputs ----
    wf_d = nc.dram_tensor("wf", [F - 1, N, RS], BF16, kind="ExternalInput")
    wb_d = nc.dram_tensor("wb", [F - 1, N, RS], BF16, kind="ExternalInput")
    bsl_d = nc.dram_tensor("bsl", [F, RS], FP32, kind="ExternalInput")
    xidx_d = nc.dram_tensor("xidx", [BC, 1], INT32, kind="ExternalInput")
    uidx_d = nc.dram_tensor("uidx", [BC, 1], INT32, kind="ExternalInput")
    out_d = nc.dram_tensor("out", [BC, 1], FP32, kind="ExternalOutput")

    # ---- internal DRAM: AllGather landing tables (x and v = u reversed) ----
    tabx = nc.dram_tensor("tabx", [F * N], FP32)   # tabx[t*N : ] = x_t
    tabv = nc.dram_tensor("tabv", [F * N], FP32)   # tabv[t*N : ] = u_{31-t}

    groups = [list(range(NCORES))]

    with tile.TileContext(nc) as tc:
        with (
            tc.tile_pool(name="const", bufs=1) as cpool,
            tc.tile_pool(name="wpool", bufs=2) as wpool,
            tc.tile_pool(name="sb", bufs=2) as sb,
            tc.tile_pool(name="ps", bufs=2, space="PSUM") as ps,
            tc.tile_pool(name="ps1", bufs=1, space="PSUM") as ps1,
            tc.tile_pool(name="agdram", bufs=2, space="DRAM") as agdram,
        ):
            # constants
            ident = cpool.tile([JT, JT], FP32)
            make_identity(nc, ident[:])
            acc = cpool.tile([JT, 128], FP32, tag="acc")
            nc.gpsimd.memset(acc[:], 0.0)
            # stationaries for the next step (produced by round t, consumed t+1)
            xT = None
            vT = None

            for t in range(F):
                # ---------- compute this round's slices ----------
                if t == 0:
                    xsl = sb.tile([1, RS], FP32, tag="xsl")
                    nc.sync.dma_start(xsl[:], bsl_d[0].unsqueeze(0))
                    vsl = sb.tile([1, RS], FP32, tag="vsl")
                    nc.gpsimd.memset(vsl[:], 1.0)
                else:
                    # fwd matvec slice: psx[0, n] = sum_j x_{t-1}[j] * W[t-1][Rc+n, j]
                    wf_t = wpool.tile([128, JT, RS], BF16, tag="wf")
                    nc.sync.dma_start(
                        wf_t[:], wf_d[t - 1].rearrange("(j p) n -> p j n", p=128)
                    )
                    psx = ps.tile([1, RS], FP32, tag="psx")
                    for j in range(JT):
                        nc.tensor.matmul(
                            psx[:], xT[:, j:j + 1], wf_t[:, j, :],
                            start=(j == 0), stop=(j == JT - 1),
                        )
                    bslab = sb.tile([1, RS], FP32, tag="bslab")
                    nc.sync.dma_start(bslab[:], bsl_d[t].unsqueeze(0))
                    xsl = sb.tile([1, RS], FP32, tag="xsl")
                    nc.vector.tensor_add(xsl[:], psx[:], bslab[:])

                # x AllGather
                aginx = agdram.tile([1, RS], FP32, tag="aginx")
                nc.sync.dma_start(aginx[:], xsl[:])
                nc.gpsimd.collective_compute(
                    "AllGather", mybir.AluOpType.bypass, replica_groups=groups,
                    ins=[aginx[:]], outs=[tabx[t * N:(t + 1) * N]],
                )

                if t > 0:
                    # bwd matvec slice: psv[0, n] = sum_i (K v)[i] * W[31-t][i, Rc+n]
                    wb_t = wpool.tile([128, JT, RS], BF16, tag="wb")
                    nc.sync.dma_start(
                        wb_t[:], wb_d[t - 1].rearrange("(j p) n -> p j n", p=128)
                    )
                    psv = ps.tile([1, RS], FP32, tag="psv")
                    for j in range(JT):
                        nc.tensor.matmul(
                            psv[:], vT[:, j:j + 1], wb_t[:, j, :],
                            start=(j == 0), stop=(j == JT - 1),
                        )
                    vsl = sb.tile([1, RS], FP32, tag="vsl")
                    nc.vector.tensor_scalar_add(vsl[:], psv[:], 1.0)

                # v AllGather
                aginv = agdram.tile([1, RS], FP32, tag="aginv")
                nc.sync.dma_start(aginv[:], vsl[:])
                nc.gpsimd.collective_compute(
                    "AllGather", mybir.AluOpType.bypass, replica_groups=groups,
                    ins=[aginv[:]], outs=[tabv[t * N:(t + 1) * N]],
                )

                # ---------- prep next stationaries from gathered vectors ----------
                x_nat = sb.tile([JT, 128], FP32, tag="x_nat")
                nc.sync.dma_start(
                    x_nat[:], tabx[t * N:(t + 1) * N].rearrange("(j l) -> j l", j=JT)
                )
                nc.vector.tensor_add(acc[:], acc[:], x_nat[:])
                if t < F - 1:
                    pstx = ps1.tile([128, JT], FP32, tag="pstx")
                    nc.tensor.transpose(pstx[:], x_nat[:], ident[:])
                    xT = sb.tile([128, JT], BF16, tag="xT")
                    nc.vector.tensor_copy(xT[:], pstx[:])

                    v_nat = sb.tile([JT, 128], FP32, tag="v_nat")
                    nc.sync.dma_start(
                        v_nat[:],
                        tabv[t * N:(t + 1) * N].rearrange("(j l) -> j l", j=JT),
                    )
                    pstv = ps1.tile([128, JT], FP32, tag="pstv")
                    nc.tensor.transpose(pstv[:], v_nat[:], ident[:])
                    vT = sb.tile([128, JT], BF16, tag="vT")
                    nc.vector.tensor_copy(vT[:], pstv[:])

            # ---------- finale: S_base broadcast + candidate gather ----------
            red = sb.tile([JT, 1], FP32, tag="red")
            nc.vector.tensor_reduce(red[:], acc[:], mybir.AxisListType.X,
                                    mybir.AluOpType.add)
            ones = cpool.tile([JT, 128], FP32, tag="ones")
            nc.gpsimd.memset(ones[:], 1.0)
            ps_sb = ps1.tile([128, 1], FP32, tag="ps_sb")
            nc.tensor.matmul(ps_sb[:], ones[:], red[:], start=True, stop=True)

            idx_x = sb.tile([BC, 1], INT32, tag="idx_x")
            idx_u = sb.tile([BC, 1], INT32, tag="idx_u")
            nc.sync.dma_start(idx_x[:], xidx_d[:])
            nc.sync.dma_start(idx_u[:], uidx_d[:])
            gx = sb.tile([BC, 1], FP32, tag="gx")
            gu = sb.tile([BC, 1], FP32, tag="gu")
            nc.gpsimd.indirect_dma_start(
                out=gx[:], out_offset=None,
                in_=tabx[:].rearrange("(a b) -> a b", b=1),
                in_offset=bass.IndirectOffsetOnAxis(ap=idx_x[:, :1], axis=0),
            )
            nc.gpsimd.indirect_dma_start(
                out=gu[:], out_offset=None,
                in_=tabv[:].rearrange("(a b) -> a b", b=1),
                in_offset=bass.IndirectOffsetOnAxis(ap=idx_u[:, :1], axis=0),
            )
            prod = sb.tile([BC, 1], FP32, tag="prod")
            nc.vector.tensor_mul(prod[:], gx[:], gu[:])
            outv = sb.tile([BC, 1], FP32, tag="outv")
            nc.vector.tensor_sub(outv[:], ps_sb[:], prod[:])
            nc.sync.dma_start(out_d[:], outv[:])

    nc.compile()
    return nc


def _get_program():
    global _PROGRAM
    if _PROGRAM is None:
        _PROGRAM = _build_program()
    return _PROGRAM


def _host_prep(weights, biases, selected_anchor_points, candidate_anchor_points):
    W = np.ascontiguousarray(weights, dtype=np.float32)
    Bi = np.ascontiguousarray(biases, dtype=np.float32)
    sel = np.asarray(selected_anchor_points)
    cand = np.asarray(candidate_anchor_points)

    K = np.ones((F, N), dtype=np.float32)
    K[sel[:, 0], sel[:, 1] * NBR + sel[:, 2]] = 0.0

    cf = cand[:, 0].astype(np.int64)
    cn = (cand[:, 1] * NBR + cand[:, 2]).astype(np.int64)
    xidx = (cf * N + cn).astype(np.int32)
    uidx = ((F - 1 - cf) * N + cn).astype(np.int32)

    import ml_dtypes
    BF = ml_dtypes.bfloat16
    # fwd: fold postmask K_t into output rows of W[t-1]  (wfm[s] = diag(K[s+1]) W[s])
    # bwd: fold premask K[32-t] into contraction rows of W[31-t]
    Wfm = W * K[1:, :, None]            # [31, N, N] rows masked by K[t]
    Wrev = W[::-1]                       # Wrev[s] = W[30-s]
    Kpre = K[F - 1:0:-1]                 # Kpre[s] = K[31-s]
    Wbm = Wrev * Kpre[:, :, None]        # [31, N, N] contraction rows masked
    bK = Bi * K                          # masked bias

    in_maps = []
    for c in range(NCORES):
        rows = slice(RS * c, RS * (c + 1))
        wf_c = np.ascontiguousarray(Wfm[:, rows, :].transpose(0, 2, 1)).astype(BF)
        wb_c = np.ascontiguousarray(Wbm[:, :, rows]).astype(BF)
        in_maps.append({
            "wf": wf_c,
            "wb": wb_c,
            "bsl": np.ascontiguousarray(bK[:, rows]),
            "xidx": xidx[BC * c: BC * (c + 1)].reshape(BC, 1),
            "uidx": uidx[BC * c: BC * (c + 1)].reshape(BC, 1),
        })
    return in_maps


def kernel(weights, biases, selected_anchor_points, candidate_anchor_points):
    nc = _get_program()
    in_maps = _host_prep(weights, biases, selected_anchor_points,
                         candidate_anchor_points)
    last_err = None
    for _attempt in range(2):
        try:
            res = run_bass_kernel_spmd(nc, in_maps,
                                       core_ids=list(range(NCORES)))
            break
        except Exception as e:  # transient device flake: retry once
            last_err = e
    else:
        raise last_err
    out = np.concatenate(
        [res.results[c]["out"].reshape(BC) for c in range(NCORES)]
    ).astype(np.float32)
    return out


# revision 7
# speedup vs baseline: 1.7334x; 1.0357x over previous
"""Trainium2 Bass kernel for nn_EstimatorNetwork (gnn_message_passing).

Mathematical reformulation: each candidate anchor (f_b, n_b) perturbs a shared
linear recurrence by a rank-1 kill, so

    total(b) = S_base - X[f_b, n_b] * U[f_b, n_b]

where X is the forward chain  x_f = K_f * (b_f + W_{f-1} @ x_{f-1})
and   U the backward chain    u_f = 1 + W_f^T (K_{f+1} * u_{f+1}),
K the selected-anchor keep mask, S_base = sum(X).

Device layout: 8 cores each own a 288-row slice of both chains. Per frame:
a weight-streaming matvec slice on the PE (vector stationary, 18 matmuls of
N=288), per-core AllGather rebuilds the full 2304-vector in a DRAM table,
PE-transpose prepares the next stationary. Final: indirect-DMA gather of the
1024 candidate (X, U) pairs + rank-1 combine.
"""
import sys

if "/opt/trn_rl_repo" not in sys.path:
    sys.path.insert(0, "/opt/trn_rl_repo")

import numpy as np

import concourse.bass as bass
import concourse.bacc as bacc
import concourse.mybir as mybir
import concourse.tile as tile
from concourse.bass_utils import run_bass_kernel_spmd
from concourse.masks import make_identity

NCORES = 8
NBR = 64            # blocks per row (node = row*64 + col)
N = 2304            # nodes per frame
F = 32              # frames
B = 1024            # candidates
RS = N // NCORES    # 288 rows per core per chain
JT = N // 128       # 18 k-tiles
BC = B // NCORES    # 128 candidates per core

FP32 = mybir.dt.float32
BF16 = mybir.dt.bfloat16
INT32 = mybir.dt.int32

_PROGRAM = None


def _build_program():
    nc = bacc.Bacc("TRN2", target_bir_lowering=False, debug=False,
                   num_devices=NCORES)

    # ---- per-core external inputs ----
    wf_d = nc.dram_tensor("wf", [F - 1, N, RS], BF16, kind="ExternalInput")
    wb_d = nc.dram_tensor("wb", [F - 1, N, RS], BF16, kind="ExternalInput")
    bsl_d = nc.dram_tensor("bsl", [F, RS], FP32, kind="ExternalInput")
    xidx_d = nc.dram_tensor("xidx", [BC, 1], INT32, kind="ExternalInput")
    uidx_d = nc.dram_tensor("uidx", [BC, 1], INT32, kind="ExternalInput")
    out_d = nc.dram_tensor("out", [BC, 1], FP32, kind="ExternalOutput")

    # ---- internal DRAM: AllGather landing tables (x and v = u reversed) ----
    tabx = nc.dram_tensor("tabx", [F * N], FP32)   # tabx[t*N : ] = x_t
    tabv = nc.dram_tensor("tabv", [F * N], FP32)   # tabv[t*N : ] = u_{31-t}

    groups = [list(range(NCORES))]

    with tile.TileContext(nc) as tc:
        with (
            tc.tile_pool(name="const", bufs=1) as cpool,
            tc.tile_pool(name="wpool", bufs=2) as wpool,
            tc.tile_pool(name="sb", bufs=2) as sb,
            tc.tile_pool(name="ps", bufs=2, space="PSUM") as ps,
            tc.tile_pool(name="ps1", bufs=1, space="PSUM") as ps1,
            tc.tile_pool(name="agdram", bufs=2, space="DRAM") as agdram,
        ):
            # constants
            ident = cpool.tile([JT, JT], FP32)
            make_identity(nc, ident[:])
            acc = cpool.tile([JT, 128], FP32, tag="acc")
            nc.gpsimd.memset(acc[:], 0.0)
            # PE keep-warm scratch (zero matmuls fill AG-wait gaps so HAM
            # stays at K=8/8)
            dum_lhs = cpool.tile([128, 1], BF16, tag="dum_lhs")
            nc.gpsimd.memset(dum_lhs[:], 0.0)
            dum_rhs = cpool.tile([128, 256], BF16, tag="dum_rhs")
            nc.gpsimd.memset(dum_rhs[:], 0.0)

            def keepalive(k):
                dps = ps1.tile([1, 256], FP32, tag="dum_ps")
                for i in range(k):
                    nc.tensor.matmul(dps[:], dum_lhs[:], dum_rhs[:],
                                     start=(i == 0), stop=(i == k - 1))

            # stationaries for the next step (produced by round t, consumed t+1)
            xT = None
            vT = None

            def prep_xT(t):
                # gathered x_t -> S_base accumulator (+ transposed stationary)
                x_nat = sb.tile([JT, 128], FP32, tag="x_nat")
                nc.scalar.dma_start(
                    x_nat[:], tabx[t * N:(t + 1) * N].rearrange("(j l) -> j l", j=JT)
                )
                nc.vector.tensor_add(acc[:], acc[:], x_nat[:])
                if t == F - 1:
                    return None
                pstx = ps1.tile([128, JT], FP32, tag="pstx")
                nc.tensor.transpose(pstx[:], x_nat[:], ident[:])
                xT = sb.tile([128, JT], BF16, tag="xT")
                nc.vector.tensor_copy(xT[:], pstx[:])
                return xT

            def prep_vT(t):
                v_nat = sb.tile([JT, 128], FP32, tag="v_nat")
                nc.scalar.dma_start(
                    v_nat[:], tabv[t * N:(t + 1) * N].rearrange("(j l) -> j l", j=JT)
                )
                pstv = ps1.tile([128, JT], FP32, tag="pstv")
                nc.tensor.transpose(pstv[:], v_nat[:], ident[:])
                vT = sb.tile([128, JT], BF16, tag="vT")
                nc.vector.tensor_copy(vT[:], pstv[:])
                return vT

            for t in range(F):
                # ---------- forward slice ----------
                if t == 0:
                    xsl = sb.tile([1, RS], FP32, tag="xsl")
                    nc.scalar.dma_start(xsl[:], bsl_d[0].unsqueeze(0))
                else:
                    # psx[0, n] = sum_j x_{t-1}[j] * Wm[t-1][Rc+n, j]
                    wf_t = wpool.tile([128, JT, RS], BF16, tag="wf")
                    nc.sync.dma_start(
                        wf_t[:], wf_d[t - 1].rearrange("(j p) n -> p j n", p=128)
                    )
                    psx = ps.tile([1, RS], FP32, tag="psx")
                    for j in range(JT):
                        nc.tensor.matmul(
                            psx[:], xT[:, j:j + 1], wf_t[:, j, :],
                            start=(j == 0), stop=(j == JT - 1),
                        )
                    bslab = sb.tile([1, RS], FP32, tag="bslab")
                    nc.scalar.dma_start(bslab[:], bsl_d[t].unsqueeze(0))
                    xsl = sb.tile([1, RS], FP32, tag="xsl")
                    nc.vector.tensor_add(xsl[:], psx[:], bslab[:])

                aginx = agdram.tile([1, RS], FP32, tag="aginx")
                nc.scalar.dma_start(aginx[:], xsl[:])
                nc.gpsimd.collective_compute(
                    "AllGather", mybir.AluOpType.bypass, replica_groups=groups,
                    ins=[aginx[:]], outs=[tabx[t * N:(t + 1) * N]],
                )

                # ---------- vT for this round's bwd matvec (gathered last round;
                # sits between the fwd and bwd MM streams on the PE queue so it
                # never blocks the fwd stream) ----------
                if t > 0:
                    vT = prep_vT(t - 1)

                # ---------- backward slice ----------
                if t == 0:
                    vsl = sb.tile([1, RS], FP32, tag="vsl")
                    nc.gpsimd.memset(vsl[:], 1.0)
                else:
                    # psv[0, n] = sum_i (K v)[i] * Wm[31-t][i, Rc+n]
                    wb_t = wpool.tile([128, JT, RS], BF16, tag="wb")
                    nc.sync.dma_start(
                        wb_t[:], wb_d[t - 1].rearrange("(j p) n -> p j n", p=128)
                    )
                    psv = ps.tile([1, RS], FP32, tag="psv")
                    for j in range(JT):
                        nc.tensor.matmul(
                            psv[:], vT[:, j:j + 1], wb_t[:, j, :],
                            start=(j == 0), stop=(j == JT - 1),
                        )
                    vsl = sb.tile([1, RS], FP32, tag="vsl")
                    nc.vector.tensor_scalar_add(vsl[:], psv[:], 1.0)

                aginv = agdram.tile([1, RS], FP32, tag="aginv")
                nc.scalar.dma_start(aginv[:], vsl[:])
                nc.gpsimd.collective_compute(
                    "AllGather", mybir.AluOpType.bypass, replica_groups=groups,
                    ins=[aginv[:]], outs=[tabv[t * N:(t + 1) * N]],
                )

                # keep PE warm while waiting for AG_x(t) to land
                keepalive(16)

                # ---------- xT for next round's fwd matvec ----------
                xT = prep_xT(t)

            # ---------- finale: S_base broadcast + candidate gather ----------
            red = sb.tile([JT, 1], FP32, tag="red")
            nc.vector.tensor_reduce(red[:], acc[:], mybir.AxisListType.X,
                                    mybir.AluOpType.add)
            ones = cpool.tile([JT, 128], FP32, tag="ones")
            nc.gpsimd.memset(ones[:], 1.0)
            ps_sb = ps1.tile([128, 1], FP32, tag="ps_sb")
            nc.tensor.matmul(ps_sb[:], ones[:], red[:], start=True, stop=True)

            idx_x = sb.tile([BC, 1], INT32, tag="idx_x")
            idx_u = sb.tile([BC, 1], INT32, tag="idx_u")
            nc.sync.dma_start(idx_x[:], xidx_d[:])
            nc.sync.dma_start(idx_u[:], uidx_d[:])
            gx = sb.tile([BC, 1], FP32, tag="gx")
            gu = sb.tile([BC, 1], FP32, tag="gu")
            nc.gpsimd.indirect_dma_start(
                out=gx[:], out_offset=None,
                in_=tabx[:].rearrange("(a b) -> a b", b=1),
                in_offset=bass.IndirectOffsetOnAxis(ap=idx_x[:, :1], axis=0),
            )
            nc.gpsimd.indirect_dma_start(
                out=gu[:], out_offset=None,
                in_=tabv[:].rearrange("(a b) -> a b", b=1),
                in_offset=bass.IndirectOffsetOnAxis(ap=idx_u[:, :1], axis=0),
            )
            prod = sb.tile([BC, 1], FP32, tag="prod")
            nc.vector.tensor_mul(prod[:], gx[:], gu[:])
            outv = sb.tile([BC, 1], FP32, tag="outv")
            nc.vector.tensor_sub(outv[:], ps_sb[:], prod[:])
            nc.sync.dma_start(out_d[:], outv[:])

    nc.compile()
    return nc


def _get_program():
    global _PROGRAM
    if _PROGRAM is None:
        _PROGRAM = _build_program()
    return _PROGRAM


def _host_prep(weights, biases, selected_anchor_points, candidate_anchor_points):
    W = np.ascontiguousarray(weights, dtype=np.float32)
    Bi = np.ascontiguousarray(biases, dtype=np.float32)
    sel = np.asarray(selected_anchor_points)
    cand = np.asarray(candidate_anchor_points)

    K = np.ones((F, N), dtype=np.float32)
    K[sel[:, 0], sel[:, 1] * NBR + sel[:, 2]] = 0.0

    cf = cand[:, 0].astype(np.int64)
    cn = (cand[:, 1] * NBR + cand[:, 2]).astype(np.int64)
    xidx = (cf * N + cn).astype(np.int32)
    uidx = ((F - 1 - cf) * N + cn).astype(np.int32)

    import ml_dtypes
    BF = ml_dtypes.bfloat16
    # fwd: fold postmask K_t into output rows of W[t-1]  (wfm[s] = diag(K[s+1]) W[s])
    # bwd: fold premask K[32-t] into contraction rows of W[31-t]
    Wfm = W * K[1:, :, None]            # [31, N, N] rows masked by K[t]
    Wrev = W[::-1]                       # Wrev[s] = W[30-s]
    Kpre = K[F - 1:0:-1]                 # Kpre[s] = K[31-s]
    Wbm = Wrev * Kpre[:, :, None]        # [31, N, N] contraction rows masked
    bK = Bi * K                          # masked bias

    in_maps = []
    for c in range(NCORES):
        rows = slice(RS * c, RS * (c + 1))
        wf_c = np.ascontiguousarray(Wfm[:, rows, :].transpose(0, 2, 1)).astype(BF)
        wb_c = np.ascontiguousarray(Wbm[:, :, rows]).astype(BF)
        in_maps.append({
            "wf": wf_c,
            "wb": wb_c,
            "bsl": np.ascontiguousarray(bK[:, rows]),
            "xidx": xidx[BC * c: BC * (c + 1)].reshape(BC, 1),
            "uidx": uidx[BC * c: BC * (c + 1)].reshape(BC, 1),
        })
    return in_maps


def kernel(weights, biases, selected_anchor_points, candidate_anchor_points):
    nc = _get_program()
    in_maps = _host_prep(weights, biases, selected_anchor_points,
                         candidate_anchor_points)
    last_err = None
    for _attempt in range(2):
        try:
            res = run_bass_kernel_spmd(nc, in_maps,
                                       core_ids=list(range(NCORES)))
            break
        except Exception as e:  # transient device flake: retry once
            last_err = e
    else:
        raise last_err
    out = np.concatenate(
        [res.results[c]["out"].reshape(BC) for c in range(NCORES)]
    ).astype(np.float32)
    return out
